# revision 1
# baseline (speedup 1.0000x reference)
"""Trainium2 Bass kernel for DenseGINConv (batch of dense graphs) — v2.

Reference computation (per graph b):
    agg  = adj[b] @ x[b]                      # [N, F_IN]
    h    = (1 + eps) * x[b] + agg
    h    = relu(h @ W1 + b1) @ W2 + b2        # 2-layer MLP per node
    out  = where(mask[b, :, None], h, 0)

Sharding: pure data parallel — B=64 graphs split 8 ways across 8 cores.

v2 vs v1: the v1 kernel was TensorE-bound (~106us PE busy: fp32 transposes
at 2 cyc/row with no fast-weight-load, fp32r small-ap fold-ins at 4 cyc/row),
not DMA-bound — measured HBM streaming on this platform is ~475 GB/s/core
for 1MB chunks and ~800 GB/s/core for 4MB chunks (far above the documented
358). v2 therefore:
  * loads adj with one 4MB SWDGE cast-DMA per graph (fp32 -> fp16 in
    flight, ~519 GB/s read-side), halving SBUF traffic and giving fp16
    on-chip operands;
  * runs all transposes/matmuls in fp16: 1 cyc/row + FWL on the weight
    load (fp32 gets neither), cutting PE busy to ~65-85us. fp16 keeps a
    10-bit mantissa (~fp32r accuracy; adj in [0,1), |h| ~ 1e2 << 65504).
  * aggregates in transposed feature space exactly as v1: aggT[f,i] over
    j-tiles with the (1+eps)*x term folded in via (1+eps)*I against the
    diagonal blocks; 2-layer MLP stays transposed; PE transposes back and
    the node mask is applied during the PSUM evict.
"""

import numpy as np
from contextlib import ExitStack

B, N, F_IN, F_HID, F_OUT = 64, 1024, 64, 128, 64
N_CORES = 8
BPC = B // N_CORES  # graphs per core
P = 128
NT = N // P  # node tiles per graph

_CACHE = {}


def _build_nc(repeat=1):
    import concourse.mybir as mybir
    import concourse.tile as tile
    from concourse import bacc
    from concourse.masks import make_identity

    fp32 = mybir.dt.float32
    fp16 = mybir.dt.float16
    AF = mybir.ActivationFunctionType

    nc = bacc.Bacc(
        "TRN2", target_bir_lowering=False, debug=False, num_devices=N_CORES
    )
    x_d = nc.declare_dram_parameter("x", [BPC, N, F_IN], fp32, isOutput=False)
    adj_d = nc.declare_dram_parameter("adj", [BPC, N, N], fp32, isOutput=False)
    mask_d = nc.declare_dram_parameter(
        "mask", [BPC, N], mybir.dt.uint8, isOutput=False
    )
    w1_d = nc.declare_dram_parameter("W1", [F_IN, F_HID], fp32, isOutput=False)
    b1_d = nc.declare_dram_parameter("b1", [F_HID, 1], fp32, isOutput=False)
    w2_d = nc.declare_dram_parameter("W2", [F_HID, F_OUT], fp32, isOutput=False)
    b2_d = nc.declare_dram_parameter("b2", [F_OUT, 1], fp32, isOutput=False)
    eps_d = nc.declare_dram_parameter("eps", [1, 1], fp32, isOutput=False)
    out_d = nc.declare_dram_parameter("out", [BPC, N, F_OUT], fp32, isOutput=True)

    with tile.TileContext(nc) as tc:
        with ExitStack() as ctx:
            const = ctx.enter_context(tc.tile_pool(name="const", bufs=1))
            ident = const.tile([P, P], fp16)
            ident32 = const.tile([P, P], fp32)
            make_identity(nc, ident32[:])
            nc.vector.tensor_copy(ident[:], ident32[:])

            w1_ld = const.tile([F_IN, F_HID], fp32)
            w1_sb = const.tile([F_IN, F_HID], fp16)
            w2_ld = const.tile([F_HID, F_OUT], fp32)
            w2_sb = const.tile([F_HID, F_OUT], fp16)
            b1_sb = const.tile([F_HID, 1], fp32)
            b2_sb = const.tile([F_OUT, 1], fp32)
            eps_sb = const.tile([1, 1], fp32)
            ones_sb = const.tile([1, P], fp32)
            c_sb = const.tile([P, 1], fp32)
            ci_sb = const.tile([P, P], fp16)

            # x for all graphs: one HWDGE DMA (fp32) + one DVE cast to fp16.
            # Layout [j-part, (b t) f].
            x_ld = const.tile([P, BPC * NT * F_IN], fp32)
            x_sb = const.tile([P, BPC * NT * F_IN], fp16)

            # mask: one 8-descriptor DMA into [BPC, N] u8, cast to fp32, then
            # NT PE transposes of [BPC, P] -> [P, BPC] assemble
            # mask_f[p, t*BPC + b] = mask[b, t*128+p].
            mask_u8 = const.tile([BPC, N], mybir.dt.uint8)
            mask_bf = const.tile([BPC, N], fp32)
            mask_f = const.tile([P, NT * BPC], fp32)

            # Working pools
            rowp = ctx.enter_context(tc.tile_pool(name="rowp", bufs=3))
            adjTp = ctx.enter_context(tc.tile_pool(name="adjTp", bufs=2))
            hp = ctx.enter_context(tc.tile_pool(name="hp", bufs=2))
            a1p = ctx.enter_context(tc.tile_pool(name="a1p", bufs=2))
            z2p = ctx.enter_context(tc.tile_pool(name="z2p", bufs=2))
            outp = ctx.enter_context(tc.tile_pool(name="outp", bufs=2))
            # PSUM (8 banks): ps_a 4x 1-bank (transpose groups), ps_o 1x
            # 1-bank (output transposes), ps_b 3x 1-bank (aggT/z1/z2).
            ps_a = ctx.enter_context(tc.tile_pool(name="ps_a", bufs=4, space="PSUM"))
            ps_o = ctx.enter_context(tc.tile_pool(name="ps_o", bufs=1, space="PSUM"))
            ps_b = ctx.enter_context(tc.tile_pool(name="ps_b", bufs=3, space="PSUM"))

            setup_done = [False]

            def setup():
                # Deferred so the small DMAs don't sit ahead of graph 0's adj
                # stream in the ring FIFOs.
                nc.sync.dma_start(out=w1_ld[:], in_=w1_d[:])
                nc.vector.tensor_copy(w1_sb[:], w1_ld[:])
                nc.sync.dma_start(out=w2_ld[:], in_=w2_d[:])
                nc.vector.tensor_copy(w2_sb[:], w2_ld[:])
                nc.sync.dma_start(out=b1_sb[:], in_=b1_d[:])
                nc.sync.dma_start(out=b2_sb[:], in_=b2_d[:])
                nc.sync.dma_start(out=eps_sb[:], in_=eps_d[:])
                nc.vector.memset(ones_sb[:], 1.0)
                # c = 1 + eps broadcast to 128 partitions via a K=1 matmul,
                # then cI = (1+eps) * I for the diagonal fold-in.
                c_ps = ps_o.tile([P, 1], fp32, tag="ps_ot")
                nc.tensor.matmul(
                    c_ps[:], ones_sb[:], eps_sb[:], start=True, stop=True
                )
                nc.scalar.add(c_sb[:], c_ps[:], 1.0)
                nc.vector.tensor_scalar_mul(ci_sb[:], ident32[:], c_sb[:, 0:1])

                nc.sync.dma_start(
                    out=x_ld[:].rearrange("p (b t f) -> p b t f", b=BPC, t=NT),
                    in_=x_d[:].rearrange("b (t p) f -> p b t f", p=P),
                )
                nc.vector.tensor_copy(x_sb[:], x_ld[:])

                nc.sync.dma_start(out=mask_u8[:], in_=mask_d[:])
                nc.vector.tensor_copy(mask_bf[:], mask_u8[:])
                for t in range(NT):
                    mt_ps = ps_o.tile([P, BPC], fp32, tag="ps_ot")
                    nc.tensor.transpose(
                        mt_ps[:],
                        mask_bf[:, t * P : (t + 1) * P],
                        ident32[0:BPC, 0:BPC],
                    )
                    nc.scalar.copy(
                        mask_f[:, t * BPC : (t + 1) * BPC], mt_ps[:]
                    )
                setup_done[0] = True

            x3 = x_sb[:].rearrange("p (b t f) -> p b t f", b=BPC, t=NT)

            for b in [g for _ in range(repeat) for g in range(BPC)]:
                # ---- adj arrives as one 4MB SWDGE cast-DMA (fp32 -> fp16)
                rows = rowp.tile([P, NT * N], fp16, tag="row")
                rows3 = rows[:].rearrange("p (r j) -> p r j", r=NT)
                nc.gpsimd.dma_start(
                    out=rows3,
                    in_=adj_d[b].rearrange("(r p) j -> p r j", p=P),
                )

                if not setup_done[0]:
                    setup()

                # ---- transpose adj into SBUF strips (PE, fp16 1 cyc/row)
                adjT = adjTp.tile([P, NT * N], fp16, tag="adjT")
                adjT3 = adjT[:].rearrange("p (j i) -> p j i", j=NT)

                # Back-end is column-independent in i; agg columns for i-half
                # ih only need adjT of row-blocks 4ih..4ih+3.
                for ih in range(2):
                    for it in (4 * ih, 4 * ih + 1, 4 * ih + 2, 4 * ih + 3):
                        for jh in range(2):
                            ps_tr = ps_a.tile([P, 4 * P], fp16, tag="ps_tr")
                            for k in range(4):
                                jt = jh * 4 + k
                                nc.tensor.transpose(
                                    ps_tr[:, k * P : (k + 1) * P],
                                    rows3[:, it, jt * P : (jt + 1) * P],
                                    ident[:],
                                )
                            dest = adjT3[
                                :, jh * 4 : (jh + 1) * 4, it * P : (it + 1) * P
                            ]
                            src = ps_tr[:].rearrange("p (k i) -> p k i", k=4)
                            if (it + jh) % 2 == 0:
                                nc.vector.tensor_copy(dest, src)
                            else:
                                nc.scalar.copy(dest, src)

                    # ---- aggregation for this i-half (contract over all jt)
                    lo = ih * 512
                    agg = ps_b.tile([F_IN, 512], fp32, tag="ps_big")
                    for jt in range(NT):
                        nc.tensor.matmul(
                            agg[:],
                            x3[:, b, jt, :],
                            adjT3[:, jt, lo : lo + 512],
                            start=(jt == 0),
                            stop=False,
                        )
                    # diagonal fold-in of (1+eps)*x
                    for k in range(4):
                        it = 4 * ih + k
                        nc.tensor.matmul(
                            agg[:, k * P : (k + 1) * P],
                            x3[:, b, it, :],
                            ci_sb[:],
                            start=False,
                            stop=True,
                        )

                    hT = hp.tile([F_IN, 512], fp16, tag="hT")
                    if ih == 0:
                        nc.vector.tensor_copy(hT[:], agg[:])
                    else:
                        nc.scalar.copy(hT[:], agg[:])

                    # ---- MLP (+relu/b1, then +b2)
                    z1 = ps_b.tile([F_HID, 512], fp32, tag="ps_big")
                    nc.tensor.matmul(z1[:], w1_sb[:], hT[:], start=True, stop=True)
                    a1 = a1p.tile([F_HID, 512], fp16, tag="a1")
                    nc.scalar.activation(a1[:], z1[:], AF.Relu, bias=b1_sb[:, 0:1])
                    z2 = ps_b.tile([F_OUT, 512], fp32, tag="ps_big")
                    nc.tensor.matmul(z2[:], w2_sb[:], a1[:], start=True, stop=True)
                    z2_sb = z2p.tile([F_OUT, 512], fp32, tag="z2_sb")
                    nc.scalar.activation(z2_sb[:], z2[:], AF.Identity, bias=b2_sb[:, 0:1])

                    # ---- transpose back + mask + store this half
                    out_sb = outp.tile([P, 4 * F_OUT], fp32, tag="out_sb")
                    ps_ot = ps_o.tile([P, 4 * F_OUT], fp32, tag="ps_ot")
                    for k in range(4):
                        it = 4 * ih + k
                        nc.tensor.transpose(
                            ps_ot[:, k * F_OUT : (k + 1) * F_OUT],
                            z2_sb[:, k * P : (k + 1) * P],
                            ident32[0:F_OUT, 0:F_OUT],
                        )
                        nc.vector.tensor_scalar_mul(
                            out_sb[:, k * F_OUT : (k + 1) * F_OUT],
                            ps_ot[:, k * F_OUT : (k + 1) * F_OUT],
                            mask_f[:, it * BPC + b : it * BPC + b + 1],
                        )
                    nc.scalar.dma_start(
                        out=out_d[b, lo : lo + 512, :].rearrange(
                            "(t p) f -> p t f", p=P
                        ),
                        in_=out_sb[:].rearrange("p (t f) -> p t f", t=4),
                    )

    nc.compile()
    return nc


def _get_nc(repeat=1):
    key = ("nc", repeat)
    if key not in _CACHE:
        _CACHE[key] = _build_nc(repeat)
    return _CACHE[key]


def _make_in_maps(inputs):
    x = np.asarray(inputs["x"], dtype=np.float32)
    adj = np.asarray(inputs["adj"], dtype=np.float32)
    mask_u8 = np.asarray(inputs["mask"]).astype(np.uint8)
    W1 = np.ascontiguousarray(np.asarray(inputs["W1"], dtype=np.float32))
    b1 = np.asarray(inputs["b1"], dtype=np.float32).reshape(F_HID, 1)
    W2 = np.ascontiguousarray(np.asarray(inputs["W2"], dtype=np.float32))
    b2 = np.asarray(inputs["b2"], dtype=np.float32).reshape(F_OUT, 1)
    eps = np.asarray(inputs["eps"], dtype=np.float32).reshape(1, 1)

    in_maps = []
    for c in range(N_CORES):
        sl = slice(c * BPC, (c + 1) * BPC)
        in_maps.append(
            {
                "x": np.ascontiguousarray(x[sl]),
                "adj": np.ascontiguousarray(adj[sl]),
                "mask": np.ascontiguousarray(mask_u8[sl]),
                "W1": W1,
                "b1": b1,
                "W2": W2,
                "b2": b2,
                "eps": eps,
            }
        )
    return in_maps


def kernel(x, adj, mask, W1, b1, W2, b2, eps):
    from concourse.bass_utils import run_bass_kernel_spmd

    nc = _get_nc()
    in_maps = _make_in_maps(
        dict(x=x, adj=adj, mask=mask, W1=W1, b1=b1, W2=W2, b2=b2, eps=eps)
    )
    res = run_bass_kernel_spmd(nc, in_maps, list(range(N_CORES)))
    out = np.concatenate(
        [res.results[c]["out"] for c in range(N_CORES)], axis=0
    )
    return out

